# revision 1
# baseline (speedup 1.0000x reference)
"""Trainium2 Bass kernel for GuidedAttention, v3.

qkv -> QK^T -> 3x3 conv+BN+sigmoid on scores -> softmax -> attn@V -> proj
-> residual. Data-parallel over batch b (8 cores).

v3 vs v2: S^T is computed per 128-row m-chunk (K=64 matmuls, cheap on PE)
into fp8 staging tiles; each conv window [16r x 8h, q] is then assembled by
ONE coalesced SBUF->SBUF DMA per window (r-major row layout r*8+h makes the
src/dst flat orders match). The conv itself runs in fp8. Everything else as
v2: 112-row et chunks with rp-major conv outputs (one scatter DMA per
window, issued on the Pool/SWDGE queue so its exp-wait cannot block the
SP gather FIFO), fp8 E/V'' so attn@V PSUM-accumulates over chunks (3
waves on the dedicated conv-PSUM pool), tanh+exp with batched exp,
gpsimd partition_broadcast renorm. PSUM layout: 2x 1-bank buffers for
S^T/V''/proj accumulators + 3x 2-bank buffers for the conv, so the conv
is triple-buffered and tanh(w) never gates conv(w+1) or conv(w+2).
"""
import sys

sys.path.insert(0, "/opt/trn_rl_repo")

import numpy as np
import ml_dtypes

import concourse.bass as bass
from concourse.ap import AP as APc
import concourse.mybir as mybir
import concourse.tile as tile
from concourse import bacc
from concourse.bass_utils import run_bass_kernel_spmd

BF16 = mybir.dt.bfloat16
F32 = mybir.dt.float32
FP8 = mybir.dt.float8e4
AF = mybir.ActivationFunctionType

N = 1024          # tokens per batch (C*h*w)
C = 512           # dim
H = 8             # heads
HD = 64           # head dim
WIN = 74          # conv windows: out rows 14w..14w+13, in rows 14w-1..14w+14
NCH = 8           # et m-chunks of 128 rows
SCH = 8           # S^T staging m-chunks of 128 rows
EXPB = 4          # windows per batched exp instruction
CONV_FP8 = True   # conv in fp8 (stg/ss fp8); else bf16 staging + conv
DR = False        # fp8 DoubleRow conv (known-broken on HW; keep off)

_O_XT = 0
_O_WQK = _O_XT + 4 * N
_O_WV = _O_WQK + 4 * 2 * C
_O_WP = _O_WV + 4 * C
_O_WCV = _O_WP + 4 * C
BLOB_COLS = _O_WCV + 3 * 112


def chunk_rows(cc):
    return 128


def build_program(reps=1, conv_fp8=None):
    conv_fp8 = CONV_FP8 if conv_fp8 is None else conv_fp8
    stg_dt = FP8 if conv_fp8 else BF16
    nc = bacc.Bacc(
        "TRN2",
        target_bir_lowering=False,
        debug=False,
        enable_asserts=False,
        num_devices=8,
    )
    blob = nc.dram_tensor("blob", [128, BLOB_COLS], BF16, kind="ExternalInput").ap()
    wcv8d = nc.dram_tensor("wcv8", [128, 3 * 112], FP8, kind="ExternalInput").ap()
    btanh = nc.dram_tensor("btanh", [112, 1], F32, kind="ExternalInput").ap()
    xres = nc.dram_tensor("xres", [128, 8 * C], F32, kind="ExternalInput").ap()
    y = nc.dram_tensor("y", [128, 8 * C], F32, kind="ExternalOutput").ap()

    with tile.TileContext(nc) as tc:
        from contextlib import ExitStack
        with ExitStack() as ctx:
            p_blob = ctx.enter_context(tc.tile_pool(name="blob", bufs=1))
            p_xres = ctx.enter_context(tc.tile_pool(name="xres", bufs=2))
            p_qkt = ctx.enter_context(tc.tile_pool(name="qkt", bufs=8))
            p_stg = ctx.enter_context(tc.tile_pool(name="stg", bufs=3))
            p_vpp = ctx.enter_context(tc.tile_pool(name="vpp", bufs=NCH))
            p_et = ctx.enter_context(tc.tile_pool(name="et", bufs=NCH))
            p_ss = ctx.enter_context(tc.tile_pool(name="ss", bufs=6))
            p_tnh = ctx.enter_context(tc.tile_pool(name="tnh", bufs=2))
            p_ei = ctx.enter_context(tc.tile_pool(name="ei", bufs=2))
            p_zt = ctx.enter_context(tc.tile_pool(name="zt", bufs=4))
            p_out = ctx.enter_context(tc.tile_pool(name="out", bufs=2))
            p_sm = ctx.enter_context(tc.tile_pool(name="small", bufs=2))
            psW = ctx.enter_context(tc.tile_pool(name="psW", bufs=2, space="PSUM"))
            psC = ctx.enter_context(tc.tile_pool(name="psC", bufs=3, space="PSUM"))

            blob_sb = p_blob.tile([128, BLOB_COLS], BF16, tag="blob")
            cut = _O_WV  # xT+wqk gate the prologue; wv/wp/wcv needed later
            nc.sync.dma_start(blob_sb[:, 0:cut], blob[:, 0:cut])
            nc.sync.dma_start(blob_sb[:, cut:], blob[:, cut:])
            xT = blob_sb[:, _O_XT:_O_XT + 4 * N]
            wqk = blob_sb[:, _O_WQK:_O_WQK + 8 * C]
            wv = blob_sb[:, _O_WV:_O_WV + 4 * C]
            wp = blob_sb[:, _O_WP:_O_WP + 4 * C]
            wcv_bf = blob_sb[:, _O_WCV:_O_WCV + 3 * 112]

            btn_sb = p_sm.tile([112, 1], F32, tag="btn")
            nc.sync.dma_start(btn_sb[:], btanh)
            half_sb = p_sm.tile([112, 1], F32, tag="half")
            nc.gpsimd.memset(half_sb[:], 0.5)
            wcv8 = p_sm.tile([128, 3 * 112], FP8, tag="wcv8")
            nc.sync.dma_start(wcv8[:], wcv8d)

            for rep in range(reps):
                # ---- prologue: Q^T (j=0..3) and K^T (j=4..7) tiles,
                # interleaved by head-pair so chunk-0 S^T can start early ----
                qkt = [None] * 8
                for j in (0, 4, 1, 5, 2, 6, 3, 7):
                    qt = p_qkt.tile([128, N], BF16, tag="qkt", name=f"qkt{j}")
                    for qc in range(2):
                        ps = (psW.tile([128, 512], F32, tag="ps", name="ps")
                              if qc == 0 else
                              psC.tile([128, N], F32, tag="pcv",
                                       name="ps")[:, 0:512])
                        for kc in range(4):
                            nc.tensor.matmul(
                                ps[:],
                                lhsT=wqk[:, kc * 2 * C + j * 128:
                                         kc * 2 * C + (j + 1) * 128],
                                rhs=xT[:, kc * N + qc * 512:
                                       kc * N + (qc + 1) * 512],
                                start=(kc == 0), stop=(kc == 3),
                            )
                        nc.vector.tensor_copy(qt[:, qc * 512:(qc + 1) * 512],
                                              ps[:])
                    qkt[j] = qt

                stg = {}      # mc -> [128, H*N] staging tile

                def stg_block(mc, b):
                    # one (h, qc) S^T block of staging chunk mc
                    if mc not in stg:
                        stg[mc] = p_stg.tile([128, H * N], stg_dt, tag="stg",
                                             name=f"stg{mc}")
                    h, qc = b // 2, b % 2
                    ps = psW.tile([128, 512], F32, tag="ps", name="ps")
                    nc.tensor.matmul(
                        ps[:],
                        lhsT=qkt[4 + h // 2][(h % 2) * 64:(h % 2) * 64 + 64,
                                             mc * 128:(mc + 1) * 128],
                        rhs=qkt[h // 2][(h % 2) * 64:(h % 2) * 64 + 64,
                                        qc * 512:(qc + 1) * 512],
                        start=True, stop=True,
                    )
                    nc.vector.tensor_copy(
                        stg[mc][:, h * N + qc * 512: h * N + (qc + 1) * 512],
                        ps[:])



                for b in range(16):
                    stg_block(0, b)

                vpp = []
                et = [p_et.tile([chunk_rows(cc), H * N], FP8, tag="et",
                                name=f"et{cc}")
                      for cc in range(NCH)]

                # ss ring: gathers only ever write cols 1..N, so the edge
                # zero-columns persist after a one-time memset per buffer
                ss_ring = [p_ss.tile([128, N + 2], stg_dt, tag="ss",
                                     name=f"ss{i}") for i in range(6)]
                for t_ in ss_ring:
                    nc.gpsimd.memset(t_[:], 0.0)

                # ---- main loop over windows ----
                tnh_t = ei_t = None
                for w in range(WIN):
                    if w % EXPB == 0:
                        tnh_t = p_tnh.tile([112, EXPB * N], BF16, tag="tnh")
                        ei_t = p_ei.tile([112, EXPB * N], FP8, tag="ei")

                    # build next staging chunk, 2 blocks per window
                    if w < 8 * (SCH - 1):
                        stg_block(w // 8 + 1, 2 * (w % 8))
                        stg_block(w // 8 + 1, 2 * (w % 8) + 1)

                    # V'' chunks interleaved (PE has slack in the loop)
                    if w % 4 == 2 and len(vpp) < NCH:
                        cc = len(vpp)
                        rows = chunk_rows(cc)
                        ps = psW.tile([128, 512], F32, tag="ps", name="ps")
                        for kc in range(4):
                            nc.tensor.matmul(
                                ps[0:rows, :],
                                lhsT=xT[:, kc * N + cc * 128:
                                        kc * N + cc * 128 + rows],
                                rhs=wv[:, kc * C:(kc + 1) * C],
                                start=(kc == 0), stop=(kc == 3),
                            )
                        vt = p_vpp.tile([rows, H * 65], FP8, tag="vpp")
                        nc.vector.tensor_copy(
                            vt[:].rearrange("p (h d) -> p h d", h=H)[:, :, 0:64],
                            ps[0:rows, :].rearrange("p (h d) -> p h d", h=H),
                        )
                        nc.gpsimd.memset(
                            vt[:].rearrange("p (h d) -> p h d", h=H)[:, :, 64:65],
                            1.0)
                        vpp.append(vt)

                    # gather the window rows: ss[r*8+h, 1+q] = S^T[14w-1+r, h, q]
                    ss = ss_ring[w % 6]
                    if w == WIN - 1:
                        # missing rows must be zero (w=0's ring buffer is
                        # still zero from the initial memset)
                        nc.gpsimd.memset(ss[:, 1:N + 1], 0.0)
                    m0 = 14 * w - 1
                    r = 0
                    while r < 16:
                        m = m0 + r
                        if m < 0 or m >= N:
                            r += 1
                            continue
                        mc = m // 128
                        rlen = 1
                        while (r + rlen < 16 and m0 + r + rlen < N
                               and (m0 + r + rlen) // 128 == mc):
                            rlen += 1
                        nc.sync.dma_start(
                            ss[8 * r: 8 * (r + rlen), 1:N + 1],
                            stg[mc][m - 128 * mc: m - 128 * mc + rlen, :],
                        )
                        r += rlen

                    # conv: 3 column-shifted banded matmuls per q-half
                    pcv = psC.tile([128, N], F32, tag="pcv", name="pcv")[0:112, :]
                    wcv = wcv8 if conv_fp8 else wcv_bf
                    for qc in range(2):
                        for dq in range(3):
                            nc.tensor.matmul(
                                pcv[:, qc * 512:(qc + 1) * 512],
                                lhsT=wcv[:, dq * 112:(dq + 1) * 112],
                                rhs=ss[:, dq + qc * 512: dq + qc * 512 + 512],
                                start=(dq == 0), stop=(dq == 2),
                            )
                    nc.scalar.activation(
                        tnh_t[:, (w % EXPB) * N:(w % EXPB + 1) * N],
                        pcv[:], AF.Tanh, bias=btn_sb[:], scale=0.5)

                    if w % EXPB == EXPB - 1 or w == WIN - 1:
                        nwin = w % EXPB + 1
                        nc.scalar.activation(ei_t[:, 0:nwin * N],
                                             tnh_t[:, 0:nwin * N],
                                             AF.Exp, bias=half_sb[:], scale=0.5)
                        for wg in range(w - nwin + 1, w + 1):
                            rows = min(14, N - 14 * wg)
                            blk = (wg % EXPB) * N
                            r_ = 0
                            while r_ < rows:
                                m_ = 14 * wg + r_
                                cc_ = m_ // 128
                                seg = min(rows - r_, 128 - m_ % 128)
                                nc.gpsimd.dma_start(
                                    et[cc_][m_ % 128: m_ % 128 + seg, :],
                                    ei_t[8 * r_: 8 * (r_ + seg),
                                         blk:blk + N],
                                )
                                r_ += seg

                # ---- attn@V: PSUM-accumulate over chunks, 3 waves on psC ----
                zt = [p_zt.tile([128, N], BF16, tag="zt", name=f"zt{i}")
                      for i in range(4)]
                for heads in ((0, 1, 2), (3, 4, 5), (6, 7)):
                    pz_tiles = {}
                    for h in heads:
                        pz = psC.tile([128, N], F32, tag="pcv",
                                      name="pz")[0:65, :]
                        pz_tiles[h] = pz
                        for qc in range(2):
                            for cc in range(NCH):
                                nc.tensor.matmul(
                                    pz[:, qc * 512:(qc + 1) * 512],
                                    lhsT=vpp[cc][:, h * 65:(h + 1) * 65],
                                    rhs=et[cc][:, h * N + qc * 512:
                                               h * N + (qc + 1) * 512],
                                    start=(cc == 0), stop=(cc == NCH - 1),
                                )
                    for h in heads:
                        pz = pz_tiles[h]
                        dvb = p_sm.tile([1, N], BF16, tag="dvb")
                        with nc.allow_low_precision("softmax denom recip bf16"):
                            nc.vector.reciprocal(dvb[:], pz[64:65, :])
                        dvbb = p_sm.tile([64, N], BF16, tag="dvbb")
                        nc.gpsimd.partition_broadcast(dvbb[:], dvb[:])
                        with nc.allow_low_precision("z renorm in bf16"):
                            nc.vector.tensor_mul(
                                zt[h // 2][(h % 2) * 64:(h % 2) * 64 + 64, :],
                                pz[0:64, :], dvbb[:])

                # ---- proj + residual ----
                for nb in range(8):
                    pp = (psW.tile([128, 512], F32, tag="ps", name="pp")
                          if nb % 2 == 0 else
                          psC.tile([128, N], F32, tag="pcv",
                                   name="pp")[:, 0:512])
                    for j in range(4):
                        nc.tensor.matmul(
                            pp[:], lhsT=zt[j][:, nb * 128:(nb + 1) * 128],
                            rhs=wp[:, j * C:(j + 1) * C],
                            start=(j == 0), stop=(j == 3),
                        )
                    xr = p_xres.tile([128, C], F32, tag="xr")
                    nc.sync.dma_start(xr[:], xres[:, nb * C:(nb + 1) * C])
                    ob = p_out.tile([128, C], F32, tag="ob")
                    nc.vector.tensor_add(ob[:], pp[:], xr[:])
                    nc.sync.dma_start(y[:, nb * C:(nb + 1) * C], ob[:])

    nc.compile()
    return nc


def host_prep(inputs, conv_fp8=None):
    bf = ml_dtypes.bfloat16
    x = np.asarray(inputs["x"], np.float32)
    qkv_w = np.asarray(inputs["qkv_w"], np.float32)
    proj_w = np.asarray(inputs["proj_w"], np.float32)
    proj_b = np.asarray(inputs["proj_b"], np.float32)
    conv_w = np.asarray(inputs["conv_w"], np.float32)
    conv_b = np.asarray(inputs["conv_b"], np.float32)
    g = np.asarray(inputs["bn_gamma"], np.float32)
    be = np.asarray(inputs["bn_beta"], np.float32)
    mu = np.asarray(inputs["bn_mean"], np.float32)
    var = np.asarray(inputs["bn_var"], np.float32)

    inv = g / np.sqrt(var + 1e-5)
    Wf = conv_w * inv[:, None, None, None]
    bpp = conv_b * inv + be - mu * inv
    Wqk = qkv_w[:2 * C].copy()
    Wqk[:C] *= HD ** -0.5

    wqk_np = np.ascontiguousarray(Wqk.T.reshape(4, 128, 2 * C))
    wv_np = np.ascontiguousarray(qkv_w[2 * C:].T.reshape(4, 128, C))
    wp_np = np.ascontiguousarray(proj_w.T.reshape(4, 128, C))

    # banded conv stationary: input rows r-major (r*8 + i), output columns
    # rp-major (rp*8 + o)
    W1 = np.zeros((3, 128, 112), np.float32)
    r = np.arange(16)
    for dq in range(3):
        for o in range(8):
            for rp in range(14):
                kw = r - rp
                m = (kw >= 0) & (kw <= 2)
                for i in range(8):
                    W1[dq, r[m] * 8 + i, rp * 8 + o] = Wf[o, i, dq, kw[m]]
    btanh_np = np.tile(0.5 * bpp, 14).reshape(112, 1).astype(np.float32)

    in_maps = []
    for core in range(8):
        x2 = x[core].reshape(N, C)
        blob = np.empty((128, BLOB_COLS), np.float32)
        blob[:, _O_XT:_O_XT + 4 * N] = x2.T.reshape(4, 128, N).transpose(
            1, 0, 2).reshape(128, 4 * N)
        blob[:, _O_WQK:_O_WQK + 8 * C] = wqk_np.transpose(1, 0, 2).reshape(128, -1)
        blob[:, _O_WV:_O_WV + 4 * C] = wv_np.transpose(1, 0, 2).reshape(128, -1)
        blob[:, _O_WP:_O_WP + 4 * C] = wp_np.transpose(1, 0, 2).reshape(128, -1)
        blob[:, _O_WCV:_O_WCV + 3 * 112] = W1.transpose(1, 0, 2).reshape(128, -1)
        in_maps.append({
            "blob": blob.astype(bf),
            "wcv8": W1.transpose(1, 0, 2).reshape(128, -1).astype(
                ml_dtypes.float8_e4m3fn),
            "btanh": btanh_np,
            "xres": np.ascontiguousarray(
                (x2 + proj_b).reshape(8, 128, C).transpose(1, 0, 2)
                .reshape(128, 8 * C)).astype(np.float32),
        })
    return in_maps


def host_post(res):
    outs = []
    for c in range(8):
        yc = res.results[c]["y"].reshape(128, 8, C).transpose(1, 0, 2)
        outs.append(yc.reshape(4, 16, 16, C))
    return np.stack(outs).astype(np.float32)


_NC_CACHE = {}


def _get_program():
    if "nc" not in _NC_CACHE:
        _NC_CACHE["nc"] = build_program()
    return _NC_CACHE["nc"]


def kernel(**inputs):
    nc = _get_program()
    in_maps = host_prep(inputs)
    res = run_bass_kernel_spmd(nc, in_maps, core_ids=list(range(8)))
    return host_post(res)

